# revision 29
# baseline (speedup 1.0000x reference)
"""DGL-MPNN layer on 8 Trainium2 NeuronCores (edge-parallel sharding).

Math: W[e] = (ef[e] @ W_edge + b_edge).reshape(64,64)
      msg[e] = nf[src[e]] @ W[e];  agg = segment_sum(msg, dst); out = agg + nf + bias

Restructured as one dense matmul per edge block:
      z[e, 64*d+h] = ef_ext[e,d] * nf[src[e],h]   (ef_ext = [ef | 1],  d=0..16)
      msg = z @ W2ext            (W2ext[64d+h, o] = W_edge[d, 64h+o]; rows 1024+: b_edge)

Per core (6250 edges, padded to 6272):
  - z^T chunks ([K=128, e] layout, chunks c=0..7 cover d=2c,2c+1) are built
    on DVE by multiplying the host-transposed gather of node features (nfT,
    [nf;nf] dup so partition p holds nf row p%64) with a host-replicated
    efrep chunk (row p = ef[2c + p//64]).  Chunk 8 (d=16, the b_edge bias
    term, ef==1) needs no multiply: its z IS nfT[0:64].
  - msg^T accumulates in PSUM with W2 chunks stationary.  The 64-wide
    output uses only half the PE array, so e-blocks are processed in
    *column-tiled pairs*: block j -> psum partitions 0:64 (tile (0,0)),
    block j+6 -> partitions 64:128 (tile (0,64)); the two matmuls run
    concurrently on the array for ~2x throughput.
  - msg^T copied PSUM->SBUF (bf16) on the scalar engine, one plain DMA out.
  - host transposes msg^T, does the segment-sum over dst and the final
    8-way reduction + residual + bias (host glue, not on the device
    critical path).
"""

import numpy as np
import ml_dtypes

N_NODES = 10000
N_EDGES = 50000
HID = 64
EDGE_DIM = 16
N_CORES = 8

E_PER = N_EDGES // N_CORES          # 6250
E_PAD = 6272                        # 49 * 128
N_CHUNKS = 9                        # chunks 0-7: K=128 (d-pairs), chunk 8: K=64 (bias)
EBLK = 512                          # msg^T moving-dim block (one PSUM bank)
N_FULL = 12                         # full 512-col blocks (12*512 = 6144)
TAIL = E_PAD - N_FULL * EBLK        # 128
N_PAIR = 6                          # block j pairs with block j+6
OUT_W = N_PAIR * EBLK + TAIL        # 3200 output cols

N_F8 = 0                            # trailing z chunks with fp8 efrep
N_BF = 8 - N_F8

BF16 = ml_dtypes.bfloat16
FP8 = ml_dtypes.float8_e4m3fn if hasattr(ml_dtypes, "float8_e4m3fn") \
    else ml_dtypes.float8_e4m3

_compiled = None


def _build():
    import concourse.bacc as bacc
    import concourse.mybir as mybir
    import concourse.tile as tile

    nc = bacc.Bacc("TRN2", target_bir_lowering=False, debug=False,
                   num_devices=N_CORES)
    dt = mybir.dt

    nfT_in = nc.dram_tensor("nfT", [64, E_PAD], dt.bfloat16,
                            kind="ExternalInput").ap()
    efrep = nc.dram_tensor("efrep", [N_BF * 128, E_PAD], dt.bfloat16,
                           kind="ExternalInput").ap()
    efrep8 = (nc.dram_tensor("efrep8", [N_F8 * 128, E_PAD], dt.float8e4,
                             kind="ExternalInput").ap() if N_F8 else None)
    w2 = nc.dram_tensor("w2", [N_CHUNKS * 128 * HID], dt.bfloat16,
                        kind="ExternalInput").ap()
    msgT_out = nc.dram_tensor("msgT", [128, OUT_W], dt.bfloat16,
                              kind="ExternalOutput").ap()

    with tile.TileContext(nc) as tc:
        with (
            tc.tile_pool(name="const", bufs=1) as cpool,
            tc.tile_pool(name="ef", bufs=N_BF) as ef_pool,
            tc.tile_pool(name="ef8", bufs=max(N_F8, 1)) as ef8_pool,
            tc.tile_pool(name="zt", bufs=3) as zt_pool,
            tc.tile_pool(name="big", bufs=1) as big_pool,
            tc.tile_pool(name="mm", bufs=1, space="PSUM") as ppool,
        ):
            # nfT: [nf.T ; nf.T] dup.  Only the top half comes over DMA
            # (critical path); the bottom half is an on-chip DVE copy.
            nfT = big_pool.tile([128, E_PAD], dt.bfloat16)
            nc.scalar.dma_start(nfT[0:64, :], nfT_in[:])
            w2_sb = cpool.tile([128, N_CHUNKS, HID], dt.bfloat16)
            nc.scalar.dma_start(
                w2_sb[:], w2.rearrange("(c p o) -> p c o", c=N_CHUNKS, p=128))
            nc.vector.tensor_copy(out=nfT[64:128, :], in_=nfT[0:64, :])

            msgT_sb = big_pool.tile([128, OUT_W], dt.bfloat16)

            # PSUM: banks 0-5 hold block pairs (j, j+6); bank 6 holds the
            # 128-col tail block in partitions 0:64.
            ptiles = [ppool.tile([128, EBLK], dt.float32, tag=f"mmp{j}",
                                 name=f"mmp{j}") for j in range(N_PAIR)]
            ptail = ppool.tile([64, TAIL], dt.float32, tag="mmt", name="mmt")
            pwarm = ppool.tile([64, EBLK], dt.float32, tag="warm", name="warm")

            def warm_mms(n):
                # junk matmuls into a scratch bank: keep the PE HAM window
                # busy through the DMA/DVE-gated gaps so real matmuls run
                # at 2.4 GHz instead of 1.2.
                for _ in range(n):
                    nc.tensor.matmul(out=pwarm[:], lhsT=w2_sb[:, 0, :],
                                     rhs=nfT[:, :EBLK], start=True, stop=True)

            # z^T chunks: DVE multiply with a multi-buffered efrep stream,
            # all on the sync HWDGE ring right behind nfT.
            zts = []
            for c in range(8):
                eng = nc.sync if c % 2 == 0 else nc.scalar
                if c < N_BF:
                    ef_sb = ef_pool.tile([128, E_PAD], dt.bfloat16, tag="ef")
                    eng.dma_start(ef_sb[:], efrep[c * 128:(c + 1) * 128, :])
                else:
                    ef_sb = ef8_pool.tile([128, E_PAD], dt.float8e4, tag="ef8")
                    eng.dma_start(
                        ef_sb[:], efrep8[(c - N_BF) * 128:(c - N_BF + 1) * 128, :])
                zt = zt_pool.tile([128, E_PAD], dt.bfloat16, tag="zt")
                nc.vector.tensor_tensor(
                    out=zt[:], in0=nfT[:], in1=ef_sb[:],
                    op=mybir.AluOpType.mult)
                zts.append(zt)

            def mm_chunk(c, start, stop):
                kp = 128 if c < 8 else 64
                rhs = nfT if c == 8 else zts[c]  # chunk 8: ef == 1
                for j in range(N_PAIR):
                    nc.tensor.matmul(
                        out=ptiles[j][0:64, :],
                        lhsT=w2_sb[:kp, c, :],
                        rhs=rhs[:kp, j * EBLK:(j + 1) * EBLK],
                        start=start, stop=stop)
                    nc.tensor.matmul(
                        out=ptiles[j][64:128, :],
                        lhsT=w2_sb[:kp, c, :],
                        rhs=rhs[:kp, (j + N_PAIR) * EBLK:(j + N_PAIR + 1) * EBLK],
                        start=start, stop=stop)
                nc.tensor.matmul(
                    out=ptail[:],
                    lhsT=w2_sb[:kp, c, :],
                    rhs=rhs[:kp, N_FULL * EBLK:],
                    start=start, stop=stop)

            # chunk 8 first: it only needs nfT + w2, so its matmuls double
            # as the HAM warmup while the efrep stream fills.
            mm_chunk(8, start=True, stop=False)
            warm_mms(4)
            for c in range(8):
                mm_chunk(c, start=False, stop=(c == 7))
                if c < 6:
                    warm_mms(3)
                elif c == 6:
                    warm_mms(5)

            nc.vector.memset(msgT_sb[64:128, N_PAIR * EBLK:], 0.0)
            # PSUM -> SBUF (bf16): banks 0-2 on ACT (free while DVE still
            # runs z7), rest on DVE; two output DMAs so the first piece
            # streams while the tail is copied.
            for j in range(3):
                nc.scalar.copy(out=msgT_sb[:, j * EBLK:(j + 1) * EBLK],
                               in_=ptiles[j][:])
            nc.scalar.dma_start(msgT_out[:, :3 * EBLK], msgT_sb[:, :3 * EBLK])
            nc.scalar.copy(out=msgT_sb[:, 3 * EBLK:4 * EBLK], in_=ptiles[3][:])
            nc.vector.tensor_copy(out=msgT_sb[:, 4 * EBLK:5 * EBLK],
                                  in_=ptiles[4][:])
            nc.vector.tensor_copy(out=msgT_sb[:, 5 * EBLK:6 * EBLK],
                                  in_=ptiles[5][:])
            nc.vector.tensor_copy(out=msgT_sb[0:64, N_PAIR * EBLK:],
                                  in_=ptail[:])
            nc.sync.dma_start(msgT_out[:, 3 * EBLK:], msgT_sb[:, 3 * EBLK:])

    nc.compile()
    return nc


def _get_compiled():
    global _compiled
    if _compiled is None:
        _compiled = _build()
    return _compiled


def kernel(nf, initial_ef, src, dst, W_edge, b_edge, bias):
    from concourse.bass_utils import run_bass_kernel_spmd

    nf = np.asarray(nf, dtype=np.float32)
    initial_ef = np.asarray(initial_ef, dtype=np.float32)
    src = np.asarray(src, dtype=np.int32)
    dst = np.asarray(dst, dtype=np.int32)
    W_edge = np.asarray(W_edge, dtype=np.float32)
    b_edge = np.asarray(b_edge, dtype=np.float32)
    bias = np.asarray(bias, dtype=np.float32)

    # ---- host-side shared prep ----
    nf_dup = np.concatenate([nf, nf], axis=1).astype(BF16)  # [N, 128]

    # W2 rows k = 64*d + h;  chunk c rows = k in [128c, 128c+128)
    w2ext = np.empty((17 * HID, HID), dtype=np.float32)
    w2ext[:EDGE_DIM * HID] = (
        W_edge.reshape(EDGE_DIM, HID, HID).reshape(EDGE_DIM * HID, HID))
    w2ext[EDGE_DIM * HID:] = b_edge.reshape(HID, HID)
    w2_pad = np.zeros((N_CHUNKS * 128, HID), dtype=np.float32)
    w2_pad[:17 * HID] = w2ext
    w2_flat = w2_pad.astype(BF16).reshape(-1)

    efT = np.ascontiguousarray(initial_ef.T)  # [16, E]

    in_maps = []
    for k in range(N_CORES):
        e0, e1 = k * E_PER, (k + 1) * E_PER
        src_k = src[e0:e1]

        nfT = np.zeros((64, E_PAD), dtype=BF16)
        nfT[:, :E_PER] = nf_dup[src_k, :64].T

        ef_k = np.zeros((EDGE_DIM, E_PAD), dtype=np.float32)
        ef_k[:, :E_PER] = efT[:, e0:e1]
        d_bf = 2 * N_BF
        m = {
            "nfT": nfT,
            "efrep": np.repeat(ef_k[:d_bf].astype(BF16), HID, axis=0),
            "w2": w2_flat,
        }
        if N_F8:
            m["efrep8"] = np.repeat(ef_k[d_bf:].astype(FP8), HID, axis=0)
        in_maps.append(m)

    nc = _get_compiled()
    res = run_bass_kernel_spmd(nc, in_maps, list(range(N_CORES)))

    out = nf + bias  # residual + bias; accumulate aggregated messages below
    msgT = np.empty((HID, E_PAD), dtype=np.float32)
    for k in range(N_CORES):
        o = res.results[k]["msgT"].astype(np.float32)  # [128, OUT_W]
        msgT[:, :N_PAIR * EBLK] = o[:64, :N_PAIR * EBLK]
        msgT[:, N_PAIR * EBLK:N_FULL * EBLK] = o[64:, :N_PAIR * EBLK]
        msgT[:, N_FULL * EBLK:] = o[:64, N_PAIR * EBLK:]
        msg = msgT.T[:E_PER]  # [6250, 64]
        np.add.at(out, dst[k * E_PER:(k + 1) * E_PER], msg)

    return out


# revision 30
# speedup vs baseline: 1.2192x; 1.2192x over previous
"""DGL-MPNN layer on 8 Trainium2 NeuronCores (edge-parallel sharding).

Math: W[e] = (ef[e] @ W_edge + b_edge).reshape(64,64)
      msg[e] = nf[src[e]] @ W[e];  agg = segment_sum(msg, dst); out = agg + nf + bias

Restructured as one dense matmul per edge block:
      z[e, 64*d+h] = ef_ext[e,d] * nf[src[e],h]   (ef_ext = [ef | 1],  d=0..16)
      msg = z @ W2ext            (W2ext[64d+h, o] = W_edge[d, 64h+o]; rows 1024+: b_edge)

Per core (6250 edges, padded to 6272):
  - z^T chunks ([K=128, e] layout, chunk c covers d=2c,2c+1) are built on
    DVE by multiplying the host-transposed gather of node features (nfT,
    [nf;nf] dup so partition p holds nf row p%64) with a host-replicated
    efrep chunk (row p = ef[2c + p//64]).  Chunk 8 (d=16, the b_edge bias
    term, ef==1) needs no multiply: its z IS nfT[0:64].
  - everything is pipelined at HALF-chunk granularity along the edge axis
    (cols 0:3072 / 3072:6272): the efrep stream, the DVE multiplies and
    the matmul groups, so the pipeline fills early and drains early.
  - msg^T accumulates in PSUM with W2 chunks stationary.  The 64-wide
    output uses only half the PE array, so e-blocks are processed in
    *column-tiled pairs*: block 2j -> psum bank j partitions 0:64 (tile
    (0,0)), block 2j+1 -> partitions 64:128 (tile (0,64)); the two
    matmuls run concurrently on the array for ~2x throughput.
  - junk matmuls into a scratch PSUM bank fill PE-idle gaps so the HAM
    clock gate keeps the PE at 2.4 GHz.
  - msg^T copied PSUM->SBUF (bf16) split across ACT and DVE, two plain
    DMAs out.  Host transposes msg^T, does the segment-sum over dst and
    the final 8-way reduction + residual + bias (host glue, off the
    device critical path).
"""

import numpy as np
import ml_dtypes

N_NODES = 10000
N_EDGES = 50000
HID = 64
EDGE_DIM = 16
N_CORES = 8

E_PER = N_EDGES // N_CORES          # 6250
E_PAD = 6272                        # 49 * 128
N_CHUNKS = 9                        # chunks 0-7: K=128 (d-pairs), chunk 8: K=64 (bias)
EBLK = 512                          # msg^T moving-dim block (half a PSUM bank)
N_FULL = 12                         # full 512-col blocks (12*512 = 6144)
TAIL = E_PAD - N_FULL * EBLK        # 128
N_BANK = 6                          # bank j holds blocks (2j, 2j+1)
HALF = 3 * 2 * EBLK                 # 3072: banks 0-2 / first 6 blocks
OUT_W = N_BANK * EBLK + TAIL        # 3200 output cols

BF16 = ml_dtypes.bfloat16

_compiled = None


def _build():
    import concourse.bacc as bacc
    import concourse.mybir as mybir
    import concourse.tile as tile

    nc = bacc.Bacc("TRN2", target_bir_lowering=False, debug=False,
                   num_devices=N_CORES)
    dt = mybir.dt

    nfT_in = nc.dram_tensor("nfT", [64, E_PAD], dt.bfloat16,
                            kind="ExternalInput").ap()
    efrep = nc.dram_tensor("efrep", [1024, E_PAD], dt.bfloat16,
                           kind="ExternalInput").ap()
    w2 = nc.dram_tensor("w2", [N_CHUNKS * 128 * HID], dt.bfloat16,
                        kind="ExternalInput").ap()
    msgT_out = nc.dram_tensor("msgT", [128, OUT_W], dt.bfloat16,
                              kind="ExternalOutput").ap()

    halves = ((0, HALF), (HALF, E_PAD))

    with tile.TileContext(nc) as tc:
        with (
            tc.tile_pool(name="const", bufs=1) as cpool,
            tc.tile_pool(name="ef", bufs=8) as ef_pool,
            tc.tile_pool(name="zt", bufs=4) as zt_pool,
            tc.tile_pool(name="big", bufs=1) as big_pool,
            tc.tile_pool(name="mm", bufs=1, space="PSUM") as ppool,
        ):
            # nfT: [nf.T ; nf.T] dup.  Only the top half comes over DMA
            # (critical path); the bottom half is an on-chip DVE copy.
            nfT = big_pool.tile([128, E_PAD], dt.bfloat16)
            nc.sync.dma_start(nfT[0:64, :], nfT_in[:])
            w2_sb = cpool.tile([128, N_CHUNKS, HID], dt.bfloat16)
            nc.scalar.dma_start(
                w2_sb[:], w2.rearrange("(c p o) -> p c o", c=N_CHUNKS, p=128))
            nc.vector.tensor_copy(out=nfT[64:128, :], in_=nfT[0:64, :])

            msgT_sb = big_pool.tile([128, OUT_W], dt.bfloat16)

            ptiles = [ppool.tile([128, EBLK], dt.float32, tag=f"mmp{j}",
                                 name=f"mmp{j}") for j in range(N_BANK)]
            ptail = ppool.tile([64, TAIL], dt.float32, tag="mmt", name="mmt")
            pwarm = ppool.tile([64, EBLK], dt.float32, tag="warm", name="warm")

            def warm_mms(n):
                for _ in range(n):
                    nc.tensor.matmul(out=pwarm[:], lhsT=w2_sb[:, 0, :],
                                     rhs=nfT[:, :EBLK], start=True, stop=True)

            # z^T chunks, built half-by-half behind the efrep stream.
            zts = []
            for c in range(8):
                ef_sb = ef_pool.tile([128, E_PAD], dt.bfloat16, tag="ef")
                zt = zt_pool.tile([128, E_PAD], dt.bfloat16, tag="zt")
                for h0, h1 in halves:
                    nc.sync.dma_start(ef_sb[:, h0:h1],
                                      efrep[c * 128:(c + 1) * 128, h0:h1])
                    nc.vector.tensor_tensor(
                        out=zt[:, h0:h1], in0=nfT[:, h0:h1],
                        in1=ef_sb[:, h0:h1], op=mybir.AluOpType.mult)
                zts.append(zt)

            def mm_half(c, h, start, stop):
                kp = 128 if c < 8 else 64
                rhs = nfT if c == 8 else zts[c]  # chunk 8: ef == 1
                for j in (range(3) if h == 0 else range(3, N_BANK)):
                    b0 = 2 * j * EBLK
                    nc.tensor.matmul(
                        out=ptiles[j][0:64, :],
                        lhsT=w2_sb[:kp, c, :],
                        rhs=rhs[:kp, b0:b0 + EBLK],
                        start=start, stop=stop)
                    nc.tensor.matmul(
                        out=ptiles[j][64:128, :],
                        lhsT=w2_sb[:kp, c, :],
                        rhs=rhs[:kp, b0 + EBLK:b0 + 2 * EBLK],
                        start=start, stop=stop)
                if h == 1:
                    nc.tensor.matmul(
                        out=ptail[:],
                        lhsT=w2_sb[:kp, c, :],
                        rhs=rhs[:kp, N_FULL * EBLK:],
                        start=start, stop=stop)

            # chunk 8 first: it only needs nfT[0:64] + w2, so its matmuls
            # double as the HAM warmup while the efrep stream fills.
            mm_half(8, 0, start=True, stop=False)
            mm_half(8, 1, start=True, stop=False)
            warm_mms(4)
            for c in range(8):
                mm_half(c, 0, start=False, stop=(c == 7))
                mm_half(c, 1, start=False, stop=(c == 7))
                if c < 6:
                    warm_mms(3)
                elif c == 6:
                    warm_mms(5)

            nc.vector.memset(msgT_sb[64:128, N_BANK * EBLK:], 0.0)
            # PSUM -> SBUF (bf16): banks 0-2 on ACT (they finish while DVE
            # still runs the last multiply), rest on DVE; two output DMAs.
            for j in range(3):
                nc.scalar.copy(out=msgT_sb[:, j * EBLK:(j + 1) * EBLK],
                               in_=ptiles[j][:])
            nc.scalar.dma_start(msgT_out[:, :3 * EBLK], msgT_sb[:, :3 * EBLK])
            nc.scalar.copy(out=msgT_sb[:, 3 * EBLK:4 * EBLK], in_=ptiles[3][:])
            for j in range(4, N_BANK):
                nc.vector.tensor_copy(out=msgT_sb[:, j * EBLK:(j + 1) * EBLK],
                                      in_=ptiles[j][:])
            nc.vector.tensor_copy(out=msgT_sb[0:64, N_BANK * EBLK:],
                                  in_=ptail[:])
            nc.sync.dma_start(msgT_out[:, 3 * EBLK:], msgT_sb[:, 3 * EBLK:])

    nc.compile()
    return nc


def _get_compiled():
    global _compiled
    if _compiled is None:
        _compiled = _build()
    return _compiled


def kernel(nf, initial_ef, src, dst, W_edge, b_edge, bias):
    from concourse.bass_utils import run_bass_kernel_spmd

    nf = np.asarray(nf, dtype=np.float32)
    initial_ef = np.asarray(initial_ef, dtype=np.float32)
    src = np.asarray(src, dtype=np.int32)
    dst = np.asarray(dst, dtype=np.int32)
    W_edge = np.asarray(W_edge, dtype=np.float32)
    b_edge = np.asarray(b_edge, dtype=np.float32)
    bias = np.asarray(bias, dtype=np.float32)

    # ---- host-side shared prep ----
    nf_bf = nf.astype(BF16)

    # W2 rows k = 64*d + h;  chunk c rows = k in [128c, 128c+128)
    w2ext = np.empty((17 * HID, HID), dtype=np.float32)
    w2ext[:EDGE_DIM * HID] = (
        W_edge.reshape(EDGE_DIM, HID, HID).reshape(EDGE_DIM * HID, HID))
    w2ext[EDGE_DIM * HID:] = b_edge.reshape(HID, HID)
    w2_pad = np.zeros((N_CHUNKS * 128, HID), dtype=np.float32)
    w2_pad[:17 * HID] = w2ext
    w2_flat = w2_pad.astype(BF16).reshape(-1)

    efT = np.ascontiguousarray(initial_ef.T)  # [16, E]

    in_maps = []
    for k in range(N_CORES):
        e0, e1 = k * E_PER, (k + 1) * E_PER
        src_k = src[e0:e1]

        nfT = np.zeros((64, E_PAD), dtype=BF16)
        nfT[:, :E_PER] = nf_bf[src_k].T

        ef_k = np.zeros((EDGE_DIM, E_PAD), dtype=np.float32)
        ef_k[:, :E_PER] = efT[:, e0:e1]
        in_maps.append({
            "nfT": nfT,
            "efrep": np.repeat(ef_k.astype(BF16), HID, axis=0),
            "w2": w2_flat,
        })

    nc = _get_compiled()
    res = run_bass_kernel_spmd(nc, in_maps, list(range(N_CORES)))

    out = nf + bias  # residual + bias; accumulate aggregated messages below
    msgT = np.empty((HID, E_PAD), dtype=np.float32)
    for k in range(N_CORES):
        o = res.results[k]["msgT"].astype(np.float32)  # [128, OUT_W]
        for b in range(N_FULL):
            lo = 64 * (b % 2)
            msgT[:, b * EBLK:(b + 1) * EBLK] = \
                o[lo:lo + 64, (b // 2) * EBLK:(b // 2 + 1) * EBLK]
        msgT[:, N_FULL * EBLK:] = o[:64, N_BANK * EBLK:]
        msg = msgT.T[:E_PER]  # [6250, 64]
        np.add.at(out, dst[k * E_PER:(k + 1) * E_PER], msg)

    return out
